# revision 14
# baseline (speedup 1.0000x reference)
"""Trainium2 Bass kernel for nn_BinarizedCifar10MLP.

Data-parallel over batch (8192/8 = 1024 rows per core), feature-major
activation layout [features, batch] on device.  BN batch statistics are
all-reduced across the 8 cores, CHUNKED per layer: an early AllReduce
covering the first MCUT m-tiles is issued while the layer's trailing
m-tiles still run (hiding the collective + sign-wave latency), and a
small late AllReduce covers the rest.

Precision scheme (reference is fp32):
  - L1 (x @ sign(W1).T): x split losslessly on host into fp16 hi + lo
    pieces; both matmul'd against host-pre-signed fp16 weights in the
    same fp32 PSUM group (hi pass then lo pass so the first matmuls only
    need the hi chunks of x).
  - L2/L3: activations and weights are exact +-1 in fp8e4m3, run with
    perf_mode=DoubleRow (2 k-tiles per matmul, ~2x PE throughput); sums
    of +-1 over 2048 terms accumulate exactly in fp32 PSUM.
  - L4: y3/W4 in fp16, log-softmax in fp32.
"""

import sys

sys.path.insert(0, "/opt/trn_rl_repo")

import numpy as np
import ml_dtypes

B, D, H, C = 8192, 3 * 32 * 32, 2048, 10
EPS = 1e-5
NCORES = 8
BS = B // NCORES          # batch rows per core
KD = D // 128             # 24 k-tiles over input dim
KH = H // 128             # 16 k-tiles over hidden dim
NB = BS // 512            # 2 free-dim chunks of 512
MCUT = {1: 12, 2: 8, 3: 8}  # m-tiles covered by the early stats AllReduce

USE_PROLOGUE = False   # early-t partial accumulations across the layer boundary
USE_BCAST_MM = False   # K=1 matmul for log-softmax broadcast (else gpsimd)

_CACHE = {}


def _build(stage=7, fast=(False, False)):
    import concourse.bacc as bacc
    import concourse.mybir as mybir
    import concourse.tile as tile

    F32 = mybir.dt.float32
    F16 = mybir.dt.float16
    F8 = mybir.dt.float8e4
    DR = mybir.MatmulPerfMode.DoubleRow
    ACT = mybir.ActivationFunctionType
    ALU = mybir.AluOpType
    RG = [list(range(NCORES))]

    nc = bacc.Bacc("TRN2", target_bir_lowering=False, debug=False, num_devices=NCORES)

    # ---- I/O ----
    xhi_d = nc.dram_tensor("xpk_hi", [128, KD * BS], F16, kind="ExternalInput").ap()
    xlo_d = nc.dram_tensor("xpk_lo", [128, KD * BS], F16, kind="ExternalInput").ap()
    w1_d = nc.dram_tensor("w1pk", [H, D], F16, kind="ExternalInput").ap()
    w2_d = nc.dram_tensor("w2pk", [H, H], F8, kind="ExternalInput").ap()
    w3_d = nc.dram_tensor("w3pk", [H, H], F8, kind="ExternalInput").ap()
    CNAMES = ("b1", "g1", "bt1", "b2", "g2", "bt2", "b3", "g3", "bt3")
    cpk_d = nc.dram_tensor("cpk", [128, KH * len(CNAMES)], F32, kind="ExternalInput").ap()
    w4pk_d = nc.dram_tensor("w4pk", [128, C * KH], F32, kind="ExternalInput").ap()
    b4_d = nc.dram_tensor("c_b4", [16, 1], F32, kind="ExternalInput").ap()
    out_d = nc.dram_tensor("outT", [C, BS], F32, kind="ExternalOutput").ap()

    wl_d = {2: w2_d, 3: w3_d}
    kl = {1: KD, 2: KH, 3: KH}

    with tile.TileContext(nc) as tc:
        with (
            tc.tile_pool(name="pconst", bufs=1) as pconst,
            tc.tile_pool(name="pstat", bufs=1) as pstat,
            tc.tile_pool(name="plog", bufs=1) as plog,
            tc.tile_pool(name="pscr", bufs=2) as pscr,
            tc.tile_pool(name="pw", bufs=2) as pw,
            tc.tile_pool(name="ph", bufs=1) as ph,
            tc.tile_pool(name="pb", bufs=1) as pb,
            tc.tile_pool(name="pa", bufs=1) as pa,
            tc.tile_pool(name="pc2", bufs=1) as pc2,
            tc.tile_pool(name="ppsum", bufs=8, space="PSUM") as ppsum,
            tc.tile_pool(name="pdram", bufs=16, space="DRAM") as pdram,
        ):
            # ---- constants first: tiny transfers, Sync engine serializes DMAs
            cpk = pconst.tile([128, KH * len(CNAMES)], F32, tag="cpk")
            nc.sync.dma_start(cpk[:], cpk_d)
            cons = {name: cpk[:, i * KH:(i + 1) * KH] for i, name in enumerate(CNAMES)}
            b4s = pconst.tile([16, 1], F32, tag="b4")
            nc.sync.dma_start(b4s[:], b4_d)
            w4st = pconst.tile([128, C * KH], F32, tag="w4st")
            nc.sync.dma_start(w4st[:], w4pk_d)

            # ---- x hi/lo chunked; first chunk + first weights lead so the
            # first matmuls start within ~6us (Sync DMAs run back-to-back)
            xhi = pa.tile([128, KD * BS], F16, tag="pa", name="xhi")
            xlo = pb.tile([128, KD * BS], F16, tag="pb", name="xlo")

            def xchunk(dst, src, k0, k1):
                nc.sync.dma_start(dst[:, k0 * BS:k1 * BS], src[:, k0 * BS:k1 * BS])

            xchunk(xhi, xhi_d, 0, 2)
            w1m0 = pw.tile([128, KD * 128], F16, tag="pw", name="w16_0")
            nc.sync.dma_start(w1m0[:], w1_d[0:128, :])
            # warm-up AllReduce (absorbs first-collective overhead; result unused)
            warm = pstat.tile([128, 8], F32, tag="warm")
            nc.vector.memset(warm[:], 0.0)
            arw_i = pdram.tile([128, 8], F32, tag="arwi")
            arw_o = pdram.tile([128, 8], F32, tag="arwo")
            nc.sync.dma_start(arw_i[:], warm[:])
            nc.gpsimd.collective_compute(
                "AllReduce", ALU.add, replica_groups=RG,
                ins=[arw_i.opt()], outs=[arw_o.opt()])
            xchunk(xhi, xhi_d, 2, 6)
            xchunk(xhi, xhi_d, 6, 12)
            xchunk(xhi, xhi_d, 12, 24)
            xchunk(xlo, xlo_d, 0, 12)
            xchunk(xlo, xlo_d, 12, 24)

            ones10 = pconst.tile([16, 1], F32, tag="ones10")
            nc.vector.memset(ones10[:], 1.0)
            ones_bc = pconst.tile([1, 16], F32, tag="onesbc")
            nc.vector.memset(ones_bc[:], 1.0)
            w4f = pconst.tile([128, C * KH], F16, tag="w4f")
            nc.vector.tensor_copy(w4f[:], w4st[:])

            parts = {}

            # per-layer stats tiles (full KH width, written chunk-wise)
            def stt(l, tag):
                return pstat.tile([128, KH], F32, name=f"{tag}{l}", tag=f"{tag}{l}")

            ST = {}
            for l in (1, 2):
                if fast[l - 1]:
                    ST[l] = {k: stt(l, k) for k in ("m1", "negm")}
                else:
                    ST[l] = {k: stt(l, k) for k in ("rp", "c", "tthr", "s2", "sneg")}
            ST[3] = {k: stt(3, k) for k in ("rp", "c")}

            def stats_ar(l, part, mlo, mhi, need_var):
                """DMA parts cols [mlo,mhi) (+sq) to DRAM, AllReduce, DMA back."""
                ncols = 2 * (mhi - mlo)
                w = ncols * (2 if need_var else 1)
                ari = pdram.tile([128, w], F32, tag=f"ari{l}{part}")
                aro = pdram.tile([128, w], F32, tag=f"aro{l}{part}")
                nc.sync.dma_start(ari[:, 0:ncols], parts[l][:, 2 * mlo:2 * mhi])
                if need_var:
                    nc.sync.dma_start(ari[:, ncols:w],
                                      parts[l][:, 32 + 2 * mlo:32 + 2 * mhi])
                nc.gpsimd.collective_compute(
                    "AllReduce", ALU.add, replica_groups=RG,
                    ins=[ari.opt()], outs=[aro.opt()])
                g_t = pstat.tile([128, w], F32, tag=f"g{l}{part}")
                nc.sync.dma_start(g_t[:], aro[:])
                return g_t

            def stats_math(l, g_t, mlo, mhi, need_var, fastp):
                mc = mhi - mlo
                msl = slice(mlo, mhi)

                def tmp(tag):
                    return pstat.tile([128, mc], F32, name=f"{tag}{l}_{mlo}",
                                      tag=f"{tag}{l}_{mlo}")

                sg = tmp("sg")
                nc.vector.tensor_reduce(
                    sg[:], g_t[:, 0:2 * mc].rearrange("p (m n) -> p m n", n=2),
                    axis=mybir.AxisListType.X, op=ALU.add)
                if fastp:
                    nc.vector.tensor_scalar_mul(ST[l]["m1"][:, msl], sg[:], 1.0 / B)
                    nc.vector.tensor_scalar_mul(ST[l]["negm"][:, msl], sg[:], -1.0 / B)
                    return
                qg, m1, msq, m1sq, v, sq, r = (
                    tmp(x) for x in ("qg", "m1", "msq", "m1sq", "v", "sq", "r"))
                nc.vector.tensor_reduce(
                    qg[:], g_t[:, 2 * mc:4 * mc].rearrange("p (m n) -> p m n", n=2),
                    axis=mybir.AxisListType.X, op=ALU.add)
                nc.vector.tensor_scalar_mul(m1[:], sg[:], 1.0 / B)
                nc.vector.tensor_scalar_mul(msq[:], qg[:], 1.0 / B)
                nc.vector.tensor_tensor(m1sq[:], m1[:], m1[:], op=ALU.mult)
                nc.vector.tensor_tensor(v[:], msq[:], m1sq[:], op=ALU.subtract)
                nc.vector.tensor_scalar_add(v[:], v[:], EPS)
                nc.scalar.activation(sq[:], v[:], ACT.Sqrt)
                nc.vector.reciprocal(r[:], sq[:])
                nc.vector.tensor_tensor(ST[l]["rp"][:, msl], cons[f"g{l}"][:, msl],
                                        r[:], op=ALU.mult)
                mt = tmp("mt")
                nc.vector.tensor_tensor(mt[:], m1[:], ST[l]["rp"][:, msl], op=ALU.mult)
                nc.vector.tensor_tensor(ST[l]["c"][:, msl], cons[f"bt{l}"][:, msl],
                                        mt[:], op=ALU.subtract)
                if l == 3:
                    return
                # DVE-path sign params: threshold t = m - bt/(g*r), steps 2s/-s
                gi, u, u2, s = (tmp(x) for x in ("gi", "u", "u2", "s"))
                nc.vector.reciprocal(gi[:], cons[f"g{l}"][:, msl])
                nc.vector.tensor_tensor(u[:], cons[f"bt{l}"][:, msl], gi[:], op=ALU.mult)
                nc.vector.tensor_tensor(u2[:], u[:], sq[:], op=ALU.mult)
                nc.vector.tensor_tensor(ST[l]["tthr"][:, msl], m1[:], u2[:],
                                        op=ALU.subtract)
                nc.scalar.activation(s[:], cons[f"g{l}"][:, msl], ACT.Sign)
                nc.vector.tensor_scalar_mul(ST[l]["s2"][:, msl], s[:], 2.0)
                nc.vector.tensor_scalar_mul(ST[l]["sneg"][:, msl], s[:], -1.0)

            def sign_chunk(l, dst, h_t, mlo, mhi, fastp, dve_only):
                """Binarize h k-chunks [mlo,mhi) into fp8 dst[:, k, :]."""
                st = ST[l]
                for k in range(mlo, mhi):
                    hsl = h_t[:, k * BS:(k + 1) * BS]
                    dk = dst[:, k, :]
                    use_act = (not dve_only) and (k % 2 == 0)
                    if use_act:
                        if fastp:
                            nc.scalar.activation(dk, hsl, ACT.Sign,
                                                 bias=st["negm"][:, k:k + 1], scale=1.0)
                        else:
                            nc.scalar.activation(dk, hsl, ACT.Sign,
                                                 bias=st["c"][:, k:k + 1],
                                                 scale=st["rp"][:, k:k + 1])
                    else:
                        bt_ = pscr.tile([128, BS], F16, tag="scr", name=f"sgb{l}_{k}")
                        thr = st["m1"][:, k:k + 1] if fastp else st["tthr"][:, k:k + 1]
                        nc.vector.tensor_scalar(out=bt_[:], in0=hsl, scalar1=thr,
                                                scalar2=None, op0=ALU.is_ge)
                        s2a = 2.0 if fastp else st["s2"][:, k:k + 1]
                        sna = -1.0 if fastp else st["sneg"][:, k:k + 1]
                        nc.vector.tensor_scalar(out=dk, in0=bt_[:], scalar1=s2a,
                                                scalar2=sna, op0=ALU.mult, op1=ALU.add)

            def y3_chunk(dst, h_t, mlo, mhi, dve_only):
                """y3 = clip(rp3*h + c3, -1, 1) for k-chunks [mlo,mhi) -> fp16."""
                for k in range(mlo, mhi):
                    hsl = h_t[:, k * BS:(k + 1) * BS]
                    scr = pscr.tile([128, BS], F32, tag="scr", name=f"y3s_{k}")
                    if dve_only:
                        nc.vector.tensor_scalar(out=scr[:], in0=hsl,
                                                scalar1=ST[3]["rp"][:, k:k + 1],
                                                scalar2=ST[3]["c"][:, k:k + 1],
                                                op0=ALU.mult, op1=ALU.add)
                    else:
                        nc.scalar.activation(scr[:], hsl, ACT.Identity,
                                             bias=ST[3]["c"][:, k:k + 1],
                                             scale=ST[3]["rp"][:, k:k + 1])
                    nc.vector.tensor_scalar(out=dst[:, k * BS:(k + 1) * BS],
                                            in0=scr[:], scalar1=-1.0, scalar2=1.0,
                                            op0=ALU.max, op1=ALU.min)

            def dense_layer(l, rhs_hi, rhs_lo, sign_dst, fastp, need_var,
                            early_kt=0):
                """One layer of matmuls + chunked stats/sign overlap.

                l == 1: fp16 hi/lo 2D path.  l >= 2: fp8 DoubleRow 3D path.
                early_kt: k-tiles of rhs available early (input's chunk-a) --
                a prologue runs partial accumulations for m0..3 over those
                k-tiles into all 8 PSUM banks, so the PE has work while the
                previous boundary's late stats chunk is still in flight.
                Returns the SBUF h tile.
                """
                K = kl[l]
                dr = l >= 2
                h_t = ph.tile([128, KH * BS], F32, tag="ph", name=f"h{l}")
                parts_l = pstat.tile([128, 64], F32, tag=f"parts{l}", name=f"parts{l}")
                parts[l] = parts_l
                bias_t = cons[f"b{l}"]
                mcut = MCUT[l]
                # math/sign emission point: right after the AR for DVE-only fast
                # math; deferred ~4 m-tiles when the math needs ScalarE (Sqrt)
                # so it doesn't block PSUM-evicting Identity ACTs in the FIFO.
                mmath = min(mcut + (0 if (l < 3 and fastp) else 4), KH)
                g_a = None
                ET = early_kt // 2 if (dr and USE_PROLOGUE) else 0
                epss = {}
                if ET > 0:
                    for mp in range(4):
                        w8e = pw.tile([128, K * 128], F8, tag="pw", name=f"w8e_{l}_{mp}")
                        nc.sync.dma_start(w8e[:], wl_d[l][mp * 128:(mp + 1) * 128, :])
                        w8ev = w8e[:].rearrange("p (k c) -> p k c", c=128)
                        for n in range(NB):
                            ps = ppsum.tile([128, 512], F32, tag="ps",
                                            name=f"pse_{l}_{mp}_{n}")
                            epss[(mp, n)] = ps
                            for t in range(ET):
                                nc.tensor.matmul(
                                    ps[:], w8ev[:, 2 * t:2 * t + 2, :],
                                    rhs_hi[:, 2 * t:2 * t + 2, n * 512:n * 512 + 512],
                                    start=(t == 0), stop=False, perf_mode=DR)
                for m in range(KH):
                    if dr:
                        w8 = pw.tile([128, K * 128], F8, tag="pw", name=f"w8_{l}_{m}")
                        nc.sync.dma_start(w8[:], wl_d[l][m * 128:(m + 1) * 128, :])
                        w8v = w8[:].rearrange("p (k c) -> p k c", c=128)
                    elif m == 0:
                        w16 = w1m0
                    else:
                        w16 = pw.tile([128, K * 128], F16, tag="pw", name=f"w16_{m}")
                        nc.sync.dma_start(w16[:], w1_d[m * 128:(m + 1) * 128, :])
                    pss = []
                    # pass A: all n-chunks (hi pass for l==1, full for DR)
                    for n in range(NB):
                        if (m, n) in epss:
                            ps = epss[(m, n)]
                            t0 = ET
                        else:
                            ps = ppsum.tile([128, 512], F32, tag="ps", name=f"ps_{l}_{m}_{n}")
                            t0 = 0
                        pss.append(ps)
                        if dr:
                            for t in range(t0, K // 2):
                                nc.tensor.matmul(
                                    ps[:], w8v[:, 2 * t:2 * t + 2, :],
                                    rhs_hi[:, 2 * t:2 * t + 2, n * 512:n * 512 + 512],
                                    start=(t == 0), stop=(t == K // 2 - 1), perf_mode=DR)
                        else:
                            for k in range(K):
                                sl = slice(k * BS + n * 512, k * BS + n * 512 + 512)
                                nc.tensor.matmul(ps[:], w16[:, k * 128:(k + 1) * 128],
                                                 rhs_hi[:, sl], start=(k == 0), stop=False)
                    # pass B (l==1 only): lo pass, then evict
                    for n in range(NB):
                        ps = pss[n]
                        if not dr:
                            for k in range(K):
                                sl = slice(k * BS + n * 512, k * BS + n * 512 + 512)
                                nc.tensor.matmul(ps[:], w16[:, k * 128:(k + 1) * 128],
                                                 rhs_lo[:, sl], start=False, stop=(k == K - 1))
                        hs = h_t[:, m * BS + n * 512: m * BS + n * 512 + 512]
                        col = 2 * m + n
                        nc.scalar.activation(hs, ps[:], ACT.Identity,
                                             bias=bias_t[:, m:m + 1], scale=1.0,
                                             accum_out=parts_l[:, col:col + 1])
                        if need_var:
                            # fused square+reduce on DVE (keeps ScalarE free to
                            # evict PSUM; late Squares were delaying the AR-b)
                            sqt = pscr.tile([128, BS], F32, tag="scr", name=f"sq_{l}_{m}_{n}")
                            nc.vector.tensor_tensor_reduce(
                                out=sqt[:, :512], in0=hs, in1=hs, scale=1.0,
                                scalar=0.0, op0=ALU.mult, op1=ALU.add,
                                accum_out=parts_l[:, 32 + col:32 + col + 1])
                    if m == mcut - 1 and stage >= l + 1:
                        # early stats chunk: AR overlapped with the layer tail
                        g_a = stats_ar(l, "a", 0, mcut, need_var)
                    if m == mmath - 1 and stage >= l + 1:
                        stats_math(l, g_a, 0, mcut, need_var, fastp)
                        if l < 3:
                            sign_chunk(l, sign_dst, h_t, 0, mcut, fastp, dve_only=True)
                        else:
                            y3_chunk(sign_dst, h_t, 0, mcut, dve_only=True)
                # late stats chunk
                if stage >= l + 1:
                    g_b = stats_ar(l, "b", mcut, KH, need_var)
                    stats_math(l, g_b, mcut, KH, need_var, fastp)
                    if l < 3:
                        sign_chunk(l, sign_dst, h_t, mcut, KH, fastp, dve_only=False)
                    else:
                        y3_chunk(sign_dst, h_t, mcut, KH, dve_only=False)
                return h_t

            def debug_out(src_ap, cast=False):
                if cast:
                    t = pscr.tile([128, BS], F32, tag="scr", name="dbgcast")
                    nc.vector.tensor_copy(t[:C, :], src_ap)
                    src_ap = t[:C, :]
                nc.sync.dma_start(out_d[:], src_ap)

            # ===== Layer 1 =====
            a2 = pc2.tile([128, KH, BS], F8, tag="pc2", name="a2")
            h1 = dense_layer(1, xhi, xlo, a2, fast[0], need_var=not fast[0])
            if stage == 1:
                debug_out(h1[:C, :BS])
            if stage == 2:
                debug_out(a2[:C, 0, :], cast=True)

            if stage >= 3:
                # ===== Layer 2 =====
                a3 = pa.tile([128, KH, BS], F8, tag="pa", name="a3")  # reuses xhi slot
                h2 = dense_layer(2, a2, None, a3, fast[1], need_var=not fast[1],
                                 early_kt=MCUT[1] if fast[0] else 0)
                if stage == 3:
                    debug_out(a3[:C, 0, :], cast=True)

            if stage >= 4:
                # ===== Layer 3 =====
                y3 = pb.tile([128, KH * BS], F16, tag="pb", name="y3")  # reuses xlo slot
                h3 = dense_layer(3, a3, None, y3, False, need_var=True,
                                 early_kt=MCUT[2] if fast[1] else 0)
                if stage == 4:
                    debug_out(y3[:C, :BS], cast=True)

            if stage >= 5:
                # ===== Layer 4 + log-softmax =====
                logits = plog.tile([16, BS], F32, tag="logits")
                for n in range(NB):
                    ps4 = ppsum.tile([128, 512], F32, tag="ps", name=f"ps4_{n}")
                    for k in range(KH):
                        nc.tensor.matmul(ps4[:C, :], w4f[:, k * C:(k + 1) * C],
                                         y3[:, k * BS + n * 512: k * BS + n * 512 + 512],
                                         start=(k == 0), stop=(k == KH - 1))
                    nc.scalar.activation(logits[:C, n * 512:(n + 1) * 512], ps4[:C, :],
                                         ACT.Identity, bias=b4s[:C, :], scale=1.0)
                if stage == 5:
                    debug_out(logits[:C, :])

            if stage >= 6:
                e_t = pscr.tile([128, BS], F32, tag="scr", name="et")
                nc.scalar.activation(e_t[:C, :], logits[:C, :], ACT.Exp)
                for n in range(NB):
                    nsl = slice(n * 512, (n + 1) * 512)
                    ps5 = ppsum.tile([128, 512], F32, tag="ps", name=f"ps5_{n}")
                    nc.tensor.matmul(ps5[:1, :], ones10[:C, :], e_t[:C, nsl],
                                     start=True, stop=True)
                    lse_n = pscr.tile([128, BS], F32, tag="scr", name=f"lse_{n}")
                    nc.scalar.activation(lse_n[:1, :512], ps5[:1, :], ACT.Ln)
                    if USE_BCAST_MM:
                        # broadcast lse across the C partitions via a K=1 matmul
                        ps6 = ppsum.tile([128, 512], F32, tag="ps", name=f"ps6_{n}")
                        nc.tensor.matmul(ps6[:C, :], ones_bc[:1, :C], lse_n[:1, :512],
                                         start=True, stop=True)
                        bcast = ps6[:C, :]
                    else:
                        lse10 = pscr.tile([128, BS], F32, tag="scr", name=f"lse10_{n}")
                        nc.gpsimd.partition_broadcast(lse10[:C, :512], lse_n[:1, :512],
                                                      channels=C)
                        bcast = lse10[:C, :512]
                    outs_n = pscr.tile([128, BS], F32, tag="scr", name=f"outs_{n}")
                    nc.vector.tensor_tensor(outs_n[:C, :512], logits[:C, nsl],
                                            bcast, op=ALU.subtract)
                    nc.sync.dma_start(out_d[:, nsl], outs_n[:C, :512])

    nc.compile()
    return nc


def _pack_w(W, kt):
    """[out, in] sign -> packed [out, in] s.t. row m*128+p, col k*128+c =
    sign(W[m*128+c, k*128+p]) (p = contraction-within-k-tile, c = out feature)."""
    S = np.where(np.asarray(W, np.float32) >= 0, 1.0, -1.0).astype(np.float32)
    mo = S.shape[0] // 128
    return np.ascontiguousarray(
        S.reshape(mo, 128, kt, 128).transpose(0, 3, 2, 1).reshape(S.shape[0], kt * 128))


def _prep_inputs(x, W1, b1, g1, bt1, W2, b2, g2, bt2, W3, b3, g3, bt3, W4, b4):
    """Host-side sharding + layout prep (layout/permutation + lossless split + sign)."""
    def as32(a):
        return np.ascontiguousarray(np.asarray(a, dtype=np.float32))

    x = as32(x)
    shared = {
        "w1pk": _pack_w(W1, KD).astype(np.float16),
        "w2pk": _pack_w(W2, KH).astype(ml_dtypes.float8_e4m3),
        "w3pk": _pack_w(W3, KH).astype(ml_dtypes.float8_e4m3),
    }
    cvecs = (b1, g1, bt1, b2, g2, bt2, b3, g3, bt3)
    cpk = np.empty((128, KH * len(cvecs)), np.float32)
    for i, v in enumerate(cvecs):
        cpk[:, i * KH:(i + 1) * KH] = as32(v).reshape(KH, 128).T
    shared["cpk"] = cpk
    w4T = np.ascontiguousarray(as32(W4).T)          # [H, C]
    w4pk = np.empty((128, C * KH), np.float32)
    for k in range(KH):
        w4pk[:, k * C:(k + 1) * C] = w4T[k * 128:(k + 1) * 128, :]
    shared["w4pk"] = w4pk
    b4p = np.zeros((16, 1), np.float32)
    b4p[:C, 0] = as32(b4).reshape(-1)
    shared["c_b4"] = b4p

    in_maps = []
    for ci in range(NCORES):
        xT = np.ascontiguousarray(x[ci * BS:(ci + 1) * BS].T)   # [D, BS]
        hi = xT.astype(np.float16)
        lo = (xT - hi.astype(np.float32)).astype(np.float16)
        # partition-major packing: row p holds all k-chunks contiguously
        def pk(a):
            return np.ascontiguousarray(
                a.reshape(KD, 128, BS).transpose(1, 0, 2).reshape(128, KD * BS))
        m = dict(shared)
        m["xpk_hi"] = pk(hi)
        m["xpk_lo"] = pk(lo)
        in_maps.append(m)
    return in_maps


def _fast_flags(inputs):
    """Mean-only BN boundaries valid when beta==0 and gamma>0."""
    def ok(g, bt):
        g, bt = np.asarray(g), np.asarray(bt)
        return bool(not np.any(bt) and np.all(g > 0))

    return (ok(inputs["g1"], inputs["bt1"]), ok(inputs["g2"], inputs["bt2"]))


def kernel(**inputs) -> np.ndarray:
    from concourse.bass_utils import run_bass_kernel_spmd

    fast = _fast_flags(inputs)
    if _CACHE.get("fast") != fast:
        _CACHE["nc"] = _build(fast=fast)
        _CACHE["fast"] = fast
    nc = _CACHE["nc"]
    in_maps = _prep_inputs(**inputs)
    res = run_bass_kernel_spmd(nc, in_maps, list(range(NCORES)))
    out = np.concatenate([res.results[c]["outT"].T for c in range(NCORES)], axis=0)
    return out.astype(np.float32)
